# revision 1
# baseline (speedup 1.0000x reference)
"""Causal self-attention (B=4, T=2048, C=1024, H=16) on 8 trn2 NeuronCores.

Sharding: core c -> (batch b = c//2, query parity par = c%2). Each core
computes the full attention block for its batch restricted to query rows
t = par (mod 2) -- an interleaved split that load-balances the causal
triangle exactly and keeps every core's program identical (SPMD); only the
input data (xT slices, diagonal mask) differs per core.

Per-core device pipeline (all matmul inputs bf16, fp32 PSUM accumulation):
  1. qT/kT projections in transposed layout [d, t]; v in natural layout
     [t, d] augmented with a ones column per head (so the attention AV
     matmul also produces the softmax denominator Z as row 64).
  2. Attention per head-pair (two heads share the 128-partition dim):
     S^T[k,q] = K Q^T via row-packed (tile_position) matmuls, exp on the
     scalar engine (no max-subtraction: logits are O(6) for these inputs,
     fp32 exp cannot overflow), causal diagonal handled by a bf16
     multiplicative mask, AV accumulated over key tiles in PSUM.
  3. Normalization: reciprocal of Z broadcast across partitions via a
     K=1 matmul with a ones vector; y^T written in bf16.
  4. Output projection from y^T; result [1024, 1024] f32 per core.

Host side: transposes/casts inputs (layout prep is part of sharding),
scatters the interleaved query rows back, adds the output bias.
"""

import numpy as np
import ml_dtypes
from contextlib import ExitStack

import concourse.bass as bass
import concourse.bacc as bacc
import concourse.mybir as mybir
import concourse.tile as tile
from concourse import bass_utils

B, T, C, H = 4, 2048, 1024, 16
HD = C // H            # 64
NCORES = 8
TQ = T // 2            # queries per core (interleaved rows)
NCH = C // 128         # 8 contraction chunks
SCALE = 1.0 / float(np.sqrt(HD))

bf16 = mybir.dt.bfloat16
f32 = mybir.dt.float32
AF = mybir.ActivationFunctionType

_compiled = {}
last_result = None  # BassKernelResults of the most recent run (for test harness)


def _build():
    nc = bacc.Bacc("TRN2", target_bir_lowering=False, debug=False,
                   num_devices=NCORES)

    xT_d = nc.dram_tensor("xT", [C, T], bf16, kind="ExternalInput")
    xTq_d = nc.dram_tensor("xTq", [C, TQ], bf16, kind="ExternalInput")
    wqT_d = nc.dram_tensor("wqT", [C, C], bf16, kind="ExternalInput")
    wkT_d = nc.dram_tensor("wkT", [C, C], bf16, kind="ExternalInput")
    wvT_d = nc.dram_tensor("wvT", [C, C], bf16, kind="ExternalInput")
    wpT_d = nc.dram_tensor("wpT", [C, C], bf16, kind="ExternalInput")
    bq_d = nc.dram_tensor("bq2", [128, NCH], f32, kind="ExternalInput")
    bk_d = nc.dram_tensor("bk2", [128, NCH], f32, kind="ExternalInput")
    bv_d = nc.dram_tensor("bv2", [1, C], bf16, kind="ExternalInput")
    mask_d = nc.dram_tensor("mask", [1024, 512], bf16, kind="ExternalInput")
    out_d = nc.dram_tensor("out", [TQ, C], f32, kind="ExternalOutput")

    xT_v = xT_d.ap().rearrange("(a p) t -> a p t", p=128)
    xTq_v = xTq_d.ap().rearrange("(a p) t -> a p t", p=128)
    wq_v = wqT_d.ap().rearrange("(a p) o -> a p o", p=128)
    wk_v = wkT_d.ap().rearrange("(a p) o -> a p o", p=128)
    wv_v = wvT_d.ap().rearrange("(a p) o -> a p o", p=128)
    wp_v = wpT_d.ap().rearrange("(a p) o -> a p o", p=128)
    mask_v = mask_d.ap().rearrange("(a p) i -> a p i", p=128)

    with tile.TileContext(nc) as tc, ExitStack() as ctx:
        persist = ctx.enter_context(tc.tile_pool(name="persist", bufs=1))
        pp = ctx.enter_context(tc.tile_pool(name="pp", bufs=2, space="PSUM"))

        kT_sb = persist.tile([128, NCH, T], bf16)
        qT_sb = persist.tile([128, NCH, TQ], bf16)
        v_sb = persist.tile([128, 16, H, HD + 1], bf16)
        bq_sb = persist.tile([128, NCH], f32)
        bk_sb = persist.tile([128, NCH], f32)
        bv_sb = persist.tile([1, C], bf16)
        ones_m = persist.tile([1, 128], bf16)   # for v-bias broadcast matmul
        ones_r = persist.tile([128, 64], bf16)  # for 1/Z broadcast matmul

        nc.vector.memset(ones_m[:], 1.0)
        nc.vector.memset(ones_r[:], 1.0)
        nc.vector.memset(v_sb[:, :, :, HD:HD + 1], 1.0)  # aug ones column
        nc.sync.dma_start(bq_sb[:], bq_d.ap())
        nc.sync.dma_start(bk_sb[:], bk_d.ap())
        nc.sync.dma_start(bv_sb[:], bv_d.ap())

        # ---------------- Phase 1: projections ----------------
        with tc.tile_pool(name="xin", bufs=1) as xin, \
             tc.tile_pool(name="wts", bufs=2) as wts:
            xT_sb = xin.tile([128, NCH, T], bf16)
            xTq_sb = xin.tile([128, NCH, TQ], bf16)
            for c in range(NCH):
                nc.sync.dma_start(xT_sb[:, c, :], xT_v[c])
                nc.sync.dma_start(xTq_sb[:, c, :], xTq_v[c])

            # K^T = Wk @ x^T  -> [dk, t]
            wk_sb = wts.tile([128, NCH, C], bf16, tag="w")
            for c in range(NCH):
                nc.sync.dma_start(wk_sb[:, c, :], wk_v[c])
            for d in range(NCH):
                for t4 in range(T // 512):
                    ps = pp.tile([128, 512], f32, tag="pp")
                    for c in range(NCH):
                        nc.tensor.matmul(
                            ps[:], wk_sb[:, c, 128 * d:128 * d + 128],
                            xT_sb[:, c, 512 * t4:512 * t4 + 512],
                            start=(c == 0), stop=(c == NCH - 1))
                    nc.vector.tensor_scalar_add(
                        kT_sb[:, d, 512 * t4:512 * t4 + 512], ps[:],
                        bk_sb[:, d:d + 1])

            # Q^T = Wq @ xq^T -> [dq, tq]
            wq_sb = wts.tile([128, NCH, C], bf16, tag="w")
            for c in range(NCH):
                nc.sync.dma_start(wq_sb[:, c, :], wq_v[c])
            for d in range(NCH):
                for t2 in range(TQ // 512):
                    ps = pp.tile([128, 512], f32, tag="pp")
                    for c in range(NCH):
                        nc.tensor.matmul(
                            ps[:], wq_sb[:, c, 128 * d:128 * d + 128],
                            xTq_sb[:, c, 512 * t2:512 * t2 + 512],
                            start=(c == 0), stop=(c == NCH - 1))
                    nc.vector.tensor_scalar_add(
                        qT_sb[:, d, 512 * t2:512 * t2 + 512], ps[:],
                        bq_sb[:, d:d + 1])

            # V = x @ Wv^T (natural layout [t, dv]) + ones column
            wv_sb = wts.tile([128, NCH, C], bf16, tag="w")
            for c in range(NCH):
                nc.sync.dma_start(wv_sb[:, c, :], wv_v[c])
            for r in range(T // 128):
                for vc in range(C // 512):
                    ps = pp.tile([128, 512], f32, tag="pp")
                    for c in range(NCH):
                        nc.tensor.matmul(
                            ps[:], xT_sb[:, c, 128 * r:128 * r + 128],
                            wv_sb[:, c, 512 * vc:512 * vc + 512],
                            start=(c == 0), stop=False)
                    nc.tensor.matmul(  # += 1 (x) bv  (bias broadcast)
                        ps[:], ones_m[:],
                        bv_sb[:, 512 * vc:512 * vc + 512],
                        start=False, stop=True)
                    nc.vector.tensor_copy(
                        v_sb[:, r, 8 * vc:8 * vc + 8, 0:HD],
                        ps[:].rearrange("p (h e) -> p h e", e=HD))

        # ---------------- Phase 2: attention ----------------
        with tc.tile_pool(name="att", bufs=1) as att, \
             tc.tile_pool(name="ppool", bufs=3) as ppool, \
             tc.tile_pool(name="spool", bufs=2, space="PSUM") as spool, \
             tc.tile_pool(name="opool", bufs=1, space="PSUM") as opool, \
             tc.tile_pool(name="small", bufs=4) as small, \
             tc.tile_pool(name="outp", bufs=3) as outp:
            mask_sb = att.tile([128, 8, 512], bf16)
            for m in range(8):
                nc.sync.dma_start(mask_sb[:, m, :], mask_v[m])
            yT_sb = att.tile([128, NCH, TQ], bf16)   # UNnormalized y^T
            wp_sb = att.tile([128, NCH, C], bf16)
            for c in range(NCH):
                nc.sync.dma_start(wp_sb[:, c, :], wp_v[c])
            zst = att.tile([128, 8, 512], f32)   # Z at partitions 0/32/64/96
            nc.vector.memset(zst[:], 1.0)        # keep recip off garbage

            for hp in range(H // 2):
                for J in range(2):
                    E = 8 * (J + 1)          # causal extent in 128-key tiles
                    qs = slice(512 * J, 512 * J + 512)
                    oA = opool.tile([HD + 1, 512], f32, tag="oA")
                    oB = opool.tile([HD + 1, 512], f32, tag="oB")
                    pend = None
                    for kt in range(E):
                        ks = slice(128 * kt, 128 * kt + 128)
                        # first valid query column in this kv tile (diag blocks)
                        i0 = 64 * (kt - 8 * J) if kt >= 8 * J else 0
                        s2 = spool.tile([128, 1024], f32, tag="s2")  # 2 banks
                        nc.tensor.matmul(s2[:, i0:512], kT_sb[0:64, hp, ks],
                                         qT_sb[0:64, hp,
                                               512 * J + i0:512 * J + 512],
                                         tile_position=(0, 0))
                        nc.tensor.matmul(s2[:, 512 + i0:1024],
                                         kT_sb[64:128, hp, ks],
                                         qT_sb[64:128, hp,
                                               512 * J + i0:512 * J + 512],
                                         tile_position=(64, 0))
                        p2 = ppool.tile([128, 1024], bf16, tag="p2")
                        s2v = s2[:].rearrange("p (h q) -> p h q", q=512)
                        p2v = p2[:].rearrange("p (h q) -> p h q", q=512)
                        nc.scalar.activation(p2v[:, :, i0:512], s2v[:, :, i0:512],
                                             AF.Exp, scale=SCALE)
                        if kt >= 8 * J:  # diagonal block: causal mask
                            m = kt - 8 * J
                            nc.vector.tensor_mul(p2[:, i0:512], p2[:, i0:512],
                                                 mask_sb[:, m, i0:512])
                            nc.vector.tensor_mul(p2[:, 512 + i0:1024],
                                                 p2[:, 512 + i0:1024],
                                                 mask_sb[:, m, i0:512])
                        if pend is not None:
                            kp, pp2, j0 = pend
                            nc.tensor.matmul(oA[:, j0:512],
                                             v_sb[:, kp, 2 * hp, :],
                                             pp2[:, j0:512],
                                             start=(kp == 0), stop=False)
                            nc.tensor.matmul(oB[:, j0:512],
                                             v_sb[:, kp, 2 * hp + 1, :],
                                             pp2[:, 512 + j0:1024],
                                             start=(kp == 0), stop=False)
                        pend = (kt, p2, i0)
                    kp, pp2, j0 = pend
                    nc.tensor.matmul(oA[:, j0:512], v_sb[:, kp, 2 * hp, :],
                                     pp2[:, j0:512], start=(kp == 0), stop=True)
                    nc.tensor.matmul(oB[:, j0:512], v_sb[:, kp, 2 * hp + 1, :],
                                     pp2[:, 512 + j0:1024],
                                     start=(kp == 0), stop=True)

                    # stash unnormalized y^T and Z; normalization is deferred
                    nc.vector.tensor_copy(yT_sb[0:64, hp, qs], oA[0:HD, :])
                    nc.vector.tensor_copy(yT_sb[64:128, hp, qs], oB[0:HD, :])
                    iA = 4 * hp + J
                    iB = 4 * hp + 2 + J
                    nc.vector.tensor_copy(
                        zst[32 * (iA % 4):32 * (iA % 4) + 1, iA // 4, :],
                        oA[HD:HD + 1, :])
                    nc.vector.tensor_copy(
                        zst[32 * (iB % 4):32 * (iB % 4) + 1, iB // 4, :],
                        oB[HD:HD + 1, :])

            # deferred normalization: one approx reciprocal over all Z
            zr = att.tile([128, 8, 512], f32)
            nc.vector.reciprocal_approx_fast(zr[:], zst[:])
            zrb = att.tile([128, 8, 512], bf16)
            nc.vector.tensor_copy(zrb[:], zr[:])
            for hp in range(H // 2):
                for J in range(2):
                    qs = slice(512 * J, 512 * J + 512)
                    for hh in range(2):
                        h = 2 * hp + hh
                        idx = 4 * hp + 2 * hh + J
                        b = 32 * (idx % 4)
                        bp1 = pp.tile([64, 512], f32, tag="pp")
                        nc.tensor.matmul(bp1[:], ones_r[b:b + 1, :],
                                         zrb[b:b + 1, idx // 4, :],
                                         tile_position=(b, 0))
                        pr = 64 * hh
                        nc.vector.tensor_mul(yT_sb[pr:pr + 64, hp, qs],
                                             yT_sb[pr:pr + 64, hp, qs], bp1[:])

            # ---------------- Phase 3: output projection ----------------
            for qt in range(TQ // 128):
                for co in range(C // 512):
                    ps = pp.tile([128, 512], f32, tag="pp")
                    for c in range(NCH):
                        nc.tensor.matmul(
                            ps[:], yT_sb[:, c, 128 * qt:128 * qt + 128],
                            wp_sb[:, c, 512 * co:512 * co + 512],
                            start=(c == 0), stop=(c == NCH - 1))
                    ot = outp.tile([128, 512], f32, tag="ot")
                    nc.vector.tensor_copy(ot[:], ps[:])
                    nc.sync.dma_start(
                        out_d.ap()[128 * qt:128 * qt + 128,
                                   512 * co:512 * co + 512], ot[:])

    nc.compile()
    return nc


def prep_in_maps(x, Wq, bq, Wk, bk, Wv, bv, Wp, bp):
    x = np.asarray(x, dtype=np.float32)
    Wq = np.asarray(Wq, dtype=np.float32)
    Wk = np.asarray(Wk, dtype=np.float32)
    Wv = np.asarray(Wv, dtype=np.float32)
    Wp = np.asarray(Wp, dtype=np.float32)
    bq = np.asarray(bq, dtype=np.float32)
    bk = np.asarray(bk, dtype=np.float32)
    bv = np.asarray(bv, dtype=np.float32)
    bp = np.asarray(bp, dtype=np.float32)

    bf = ml_dtypes.bfloat16
    wqT = np.ascontiguousarray(Wq.T).astype(bf)
    wkT = np.ascontiguousarray(Wk.T).astype(bf)
    wvT = np.ascontiguousarray(Wv.T).astype(bf)
    wpT = np.ascontiguousarray(Wp.T).astype(bf)
    bq2 = np.ascontiguousarray(bq.reshape(NCH, 128).T)
    bk2 = np.ascontiguousarray(bk.reshape(NCH, 128).T)
    bv2 = np.ascontiguousarray(bv.reshape(1, C)).astype(bf)

    kk = np.arange(1024)[:, None]
    ii = np.arange(512)[None, :]
    masks = [np.ascontiguousarray((kk <= 2 * ii + par).astype(bf))
             for par in range(2)]

    in_maps = []
    for core in range(NCORES):
        b, par = core // 2, core % 2
        xb = x[b]
        xT = np.ascontiguousarray(xb.T).astype(bf)
        xTq = np.ascontiguousarray(xb[par::2].T).astype(bf)
        in_maps.append({
            "xT": xT, "xTq": xTq,
            "wqT": wqT, "wkT": wkT, "wvT": wvT, "wpT": wpT,
            "bq2": bq2, "bk2": bk2, "bv2": bv2,
            "mask": masks[par],
        })
    return in_maps


def kernel(x, Wq, bq, Wk, bk, Wv, bv, Wp, bp, **_ignored):
    global last_result
    bp = np.asarray(bp, dtype=np.float32)
    in_maps = prep_in_maps(x, Wq, bq, Wk, bk, Wv, bv, Wp, bp)

    if "nc" not in _compiled:
        _compiled["nc"] = _build()
    nc = _compiled["nc"]

    last_result = bass_utils.run_bass_kernel_spmd(
        nc, in_maps, core_ids=list(range(NCORES)))

    out = np.empty((B, T, C), dtype=np.float32)
    for core in range(NCORES):
        b, par = core // 2, core % 2
        out[b, par::2, :] = last_result.results[core]["out"]
    out += bp[None, None, :]
    return out



# revision 6
# speedup vs baseline: 1.5305x; 1.5305x over previous
"""Causal self-attention (B=4, T=2048, C=1024, H=16) on 8 trn2 NeuronCores.

Sharding: core c -> (batch b = c//2, head-group g = c%2). Each core owns
heads 8g..8g+7 (feature dims 512g..512g+512) of its batch: it projects
q/k/v only for those 512 dims (no duplicated K/V work across cores),
runs attention for its 8 heads over the full causal sequence, and emits
a partial output projection; the host sums the two head-group partials
per batch and adds the output bias.

Per-core device pipeline (bf16 matmuls, fp32 PSUM accumulation):
  - Projections per head-pair hp (two heads share the 128-partition d):
    kT/qT in transposed [d, t] layout, v natural [t, d] with a ones
    column per head (AV matmul then also yields the softmax denom Z).
  - Attention per head-pair, query blocks J of 512 (natural order),
    key tiles of 128 with 128-granular causal trimming: S^T = K Q^T via
    row-packed tile_position matmuls (two heads concurrent), exp on the
    scalar engine (logits O(6), no max subtraction), diagonal tiles get
    a single 128x128 triangular multiplicative mask, AV accumulated
    over key tiles in PSUM.
  - Software pipelining: projection matmuls of head-pair hp+1 are
    interleaved into the attention kt-loop of hp so the tensor engine
    never waits on the scalar engine's exp stream.
  - Deferred normalization: 1/Z broadcast across partitions via K=1
    matmuls, applied to yT; output projection accumulates the 4 d-chunks
    and DMAs straight from PSUM.
"""

import numpy as np
import ml_dtypes
from contextlib import ExitStack

import concourse.bass as bass
import concourse.bacc as bacc
import concourse.mybir as mybir
import concourse.tile as tile
from concourse import bass_utils

B, T, C, H = 4, 2048, 1024, 16
HD = C // H            # 64
NCORES = 8
CG = C // 2            # 512 feature dims per core (8 heads)
NHP = CG // 128        # 4 head-pairs per core
NCH = C // 128         # 8 contraction chunks over C
NJ = T // 512          # 4 query blocks
SCALE = 1.0 / float(np.sqrt(HD))

bf16 = mybir.dt.bfloat16
f32 = mybir.dt.float32
AF = mybir.ActivationFunctionType

_compiled = {}
last_result = None  # BassKernelResults of the most recent run (for test harness)


def _build():
    nc = bacc.Bacc("TRN2", target_bir_lowering=False, debug=False,
                   num_devices=NCORES)

    xT_d = nc.dram_tensor("xT", [C, T], bf16, kind="ExternalInput")
    wqT_d = nc.dram_tensor("wqT", [C, CG], bf16, kind="ExternalInput")
    wkT_d = nc.dram_tensor("wkT", [C, CG], bf16, kind="ExternalInput")
    wvT_d = nc.dram_tensor("wvT", [C, CG], bf16, kind="ExternalInput")
    wpT_d = nc.dram_tensor("wpT", [CG, C], bf16, kind="ExternalInput")
    bq_d = nc.dram_tensor("bq2", [128, NHP], f32, kind="ExternalInput")
    bk_d = nc.dram_tensor("bk2", [128, NHP], f32, kind="ExternalInput")
    bv_d = nc.dram_tensor("bv2", [1, CG], bf16, kind="ExternalInput")
    mask_d = nc.dram_tensor("mask", [128, 256], bf16, kind="ExternalInput")
    out_d = nc.dram_tensor("out", [T, C], f32, kind="ExternalOutput")

    xT_v = xT_d.ap().rearrange("(a p) t -> a p t", p=128)
    wq_v = wqT_d.ap().rearrange("(a p) o -> a p o", p=128)
    wk_v = wkT_d.ap().rearrange("(a p) o -> a p o", p=128)
    wv_v = wvT_d.ap().rearrange("(a p) o -> a p o", p=128)
    wp_v = wpT_d.ap().rearrange("(a p) o -> a p o", p=128)

    with tile.TileContext(nc) as tc, ExitStack() as ctx:
        persist = ctx.enter_context(tc.tile_pool(name="persist", bufs=1))
        pp = ctx.enter_context(tc.tile_pool(name="pp", bufs=2, space="PSUM"))
        spool = ctx.enter_context(
            tc.tile_pool(name="spool", bufs=2, space="PSUM"))
        opool = ctx.enter_context(
            tc.tile_pool(name="opool", bufs=1, space="PSUM"))
        p2pool = ctx.enter_context(tc.tile_pool(name="p2pool", bufs=3))

        xT_sb = persist.tile([128, NCH, T], bf16)
        wq_sb = persist.tile([128, NCH, CG], bf16)
        wk_sb = persist.tile([128, NCH, CG], bf16)
        wv_sb = persist.tile([128, NCH, CG], bf16)
        wp_sb = persist.tile([128, NHP, C], bf16)
        kT_sb = persist.tile([128, NHP, T], bf16)
        qT_sb = persist.tile([128, NHP, T], bf16)
        v_sb = persist.tile([128, 16, 8, HD + 1], bf16)
        yT_sb = persist.tile([128, NHP, T], bf16)
        # Z for (hp, J, head): partition 64*head + 32*(hp%2), slot 4*(hp//2)+J
        zst = persist.tile([128, 8, 512], f32)
        zr = persist.tile([128, 4, 512], f32)
        bq_sb = persist.tile([128, NHP], f32)
        bk_sb = persist.tile([128, NHP], f32)
        bv_sb = persist.tile([1, CG], bf16)
        mask_sb = persist.tile([128, 2, 128], bf16)
        ones_m = persist.tile([1, 128], bf16)      # v-bias broadcast matmul
        ones_r = persist.tile([128, HD], f32)      # 1/Z broadcast matmul

        nc.vector.memset(ones_m[:], 1.0)
        nc.vector.memset(ones_r[:], 1.0)
        nc.vector.memset(v_sb[:, :, :, HD:HD + 1], 1.0)
        nc.gpsimd.memset(zst[:], 1.0)

        # input DMAs: wk/x chunks first (first projection group needs them)
        for c in range(NCH):
            nc.sync.dma_start(wk_sb[:, c, :], wk_v[c])
            nc.sync.dma_start(xT_sb[:, c, :], xT_v[c])
        for c in range(NCH):
            nc.sync.dma_start(wq_sb[:, c, :], wq_v[c])
        for c in range(NCH):
            nc.sync.dma_start(wv_sb[:, c, :], wv_v[c])
        for d in range(NHP):
            nc.sync.dma_start(wp_sb[:, d, :], wp_v[d])
        nc.sync.dma_start(bq_sb[:], bq_d.ap())
        nc.sync.dma_start(bk_sb[:], bk_d.ap())
        nc.sync.dma_start(bv_sb[:], bv_d.ap())
        nc.sync.dma_start(mask_sb[:], mask_d.ap())

        # ---------------- projection emitters (pipelined as work items) ----
        def proj_kq(w_sb, b_sb, dst_sb, hp, tb):
            """One 512-col t-block of the kT/qT projection for head-pair hp."""
            ps = pp.tile([128, 512], f32, tag="pp")
            ts = slice(512 * tb, 512 * tb + 512)
            for c in range(NCH):
                nc.tensor.matmul(
                    ps[:], w_sb[:, c, 128 * hp:128 * hp + 128],
                    xT_sb[:, c, ts],
                    start=(c == 0), stop=(c == NCH - 1))
            nc.vector.tensor_scalar_add(dst_sb[:, hp, ts], ps[:],
                                        b_sb[:, hp:hp + 1])

        def proj_v(half, r):
            """V rows [128r, 128r+128) for head-pairs {2*half, 2*half+1}."""
            ps = pp.tile([128, 512], f32, tag="pp")
            ds = slice(256 * half, 256 * half + 256)
            for c in range(NCH):
                nc.tensor.matmul(
                    ps[:, 0:256], xT_sb[:, c, 128 * r:128 * r + 128],
                    wv_sb[:, c, ds], start=(c == 0), stop=False)
            nc.tensor.matmul(ps[:, 0:256], ones_m[:], bv_sb[:, ds],
                             start=False, stop=True)
            nc.vector.tensor_copy(
                v_sb[:, r, 4 * half:4 * half + 4, 0:HD],
                ps[:, 0:256].rearrange("p (h e) -> p h e", e=HD))

        def norm_hp(hp):
            """Normalize yT[hp] by 1/Z (reciprocal + K=1 broadcast matmul)."""
            base = 4 * (hp // 2)
            pa = 32 * (hp % 2)        # Z row for head A
            pb = 64 + 32 * (hp % 2)   # Z row for head B
            nc.vector.reciprocal_approx_fast(zr[:], zst[:, base:base + 4, :])
            for J in range(NJ):
                qs = slice(512 * J, 512 * J + 512)
                bp1 = pp.tile([128, 512], f32, tag="pp")
                nc.tensor.matmul(bp1[0:64, :], ones_r[pa:pa + 1, :],
                                 zr[pa:pa + 1, J, :], tile_position=(pa, 0))
                nc.tensor.matmul(bp1[64:128, :], ones_r[pb:pb + 1, :],
                                 zr[pb:pb + 1, J, :], tile_position=(pb, 64))
                nc.vector.tensor_mul(yT_sb[:, hp, qs], yT_sb[:, hp, qs],
                                     bp1[:])

        # ---------------- prologue ----------------
        for tb in range(4):
            proj_kq(wk_sb, bk_sb, kT_sb, 0, tb)
        for tb in range(4):
            proj_kq(wq_sb, bq_sb, qT_sb, 0, tb)
        for r in range(16):
            proj_v(0, r)

        # ---------------- attention, pipelined with next projections ------
        for hp in range(NHP):
            work = []
            if hp < NHP - 1:
                for tb in range(4):
                    work.append(("kq", wk_sb, bk_sb, kT_sb, hp + 1, tb))
                for tb in range(4):
                    work.append(("kq", wq_sb, bq_sb, qT_sb, hp + 1, tb))
            if hp == 0:
                for r in range(16):
                    work.append(("v", 1, r))
            if hp == NHP - 1:
                for h2 in range(NHP - 1):
                    work.append(("norm", h2))
            steps = sum(4 * J + 4 for J in range(NJ))  # 40
            sched = {}
            for i, w in enumerate(work):
                s = min(steps - 1, (i * steps) // max(1, len(work)) + 1)
                sched.setdefault(s, []).append(w)
            step = 0

            for J in range(NJ):
                qs = slice(512 * J, 512 * J + 512)
                oA = opool.tile([HD + 1, 512], f32, tag="oA")
                oB = opool.tile([HD + 1, 512], f32, tag="oB")
                pend = None
                for kt in range(4 * J + 4):
                    ks = slice(128 * kt, 128 * kt + 128)
                    i0 = 128 * (kt - 4 * J) if kt >= 4 * J else 0
                    s2 = spool.tile([128, 1024], f32, tag="s2")
                    nc.tensor.matmul(
                        s2[:, i0:512], kT_sb[0:64, hp, ks],
                        qT_sb[0:64, hp, 512 * J + i0:512 * J + 512],
                        tile_position=(0, 0))
                    nc.tensor.matmul(
                        s2[:, 512 + i0:1024], kT_sb[64:128, hp, ks],
                        qT_sb[64:128, hp, 512 * J + i0:512 * J + 512],
                        tile_position=(64, 0))
                    p2 = p2pool.tile([128, 1024], bf16, tag="p2")
                    if i0 == 0:
                        nc.scalar.activation(p2[:], s2[:], AF.Exp, scale=SCALE)
                    else:
                        s2v = s2[:].rearrange("p (h q) -> p h q", q=512)
                        p2v = p2[:].rearrange("p (h q) -> p h q", q=512)
                        nc.scalar.activation(p2v[:, :, i0:512],
                                             s2v[:, :, i0:512],
                                             AF.Exp, scale=SCALE)
                    if kt >= 4 * J:  # diagonal tile: 128x128 triangular mask
                        p2v = p2[:].rearrange("p (h q) -> p h q", q=512)
                        nc.vector.tensor_mul(p2v[:, :, i0:i0 + 128],
                                             p2v[:, :, i0:i0 + 128],
                                             mask_sb[:])
                    if pend is not None:
                        kp, pp2, j0 = pend
                        nc.tensor.matmul(oA[:, j0:512],
                                         v_sb[:, kp, 2 * hp, :],
                                         pp2[:, j0:512],
                                         start=(kp == 0), stop=False)
                        nc.tensor.matmul(oB[:, j0:512],
                                         v_sb[:, kp, 2 * hp + 1, :],
                                         pp2[:, 512 + j0:1024],
                                         start=(kp == 0), stop=False)
                    pend = (kt, p2, i0)
                    for w in sched.get(step, ()):
                        if w[0] == "kq":
                            proj_kq(*w[1:])
                        elif w[0] == "v":
                            proj_v(w[1], w[2])
                        else:
                            norm_hp(w[1])
                    step += 1
                kp, pp2, j0 = pend
                nc.tensor.matmul(oA[:, j0:512], v_sb[:, kp, 2 * hp, :],
                                 pp2[:, j0:512], start=(kp == 0), stop=True)
                nc.tensor.matmul(oB[:, j0:512], v_sb[:, kp, 2 * hp + 1, :],
                                 pp2[:, 512 + j0:1024],
                                 start=(kp == 0), stop=True)
                # stash unnormalized y^T and Z (normalization deferred)
                nc.vector.tensor_copy(yT_sb[0:64, hp, qs], oA[0:HD, :])
                nc.vector.tensor_copy(yT_sb[64:128, hp, qs], oB[0:HD, :])
                za = 32 * (hp % 2)
                zslot = 4 * (hp // 2) + J
                nc.vector.tensor_copy(zst[za:za + 1, zslot, :],
                                      oA[HD:HD + 1, :])
                nc.vector.tensor_copy(zst[64 + za:65 + za, zslot, :],
                                      oB[HD:HD + 1, :])

        norm_hp(NHP - 1)

        # ---------------- output projection (partial; host sums groups) ---
        with tc.tile_pool(name="outp", bufs=4) as outp:
            for tt in range(16):
                for ch in range(2):
                    ps = pp.tile([128, 512], f32, tag="pp")
                    for d in range(NHP):
                        nc.tensor.matmul(
                            ps[:], yT_sb[:, d, 128 * tt:128 * tt + 128],
                            wp_sb[:, d, 512 * ch:512 * ch + 512],
                            start=(d == 0), stop=(d == NHP - 1))
                    ot = outp.tile([128, 512], f32, tag="ot")
                    if ch == 0:
                        nc.scalar.copy(ot[:], ps[:])
                    else:
                        nc.vector.tensor_copy(ot[:], ps[:])
                    nc.sync.dma_start(
                        out_d.ap()[128 * tt:128 * tt + 128,
                                   512 * ch:512 * ch + 512], ot[:])

    nc.compile()
    return nc


def prep_in_maps(x, Wq, bq, Wk, bk, Wv, bv, Wp, bp):
    x = np.asarray(x, dtype=np.float32)
    Wq = np.asarray(Wq, dtype=np.float32)
    Wk = np.asarray(Wk, dtype=np.float32)
    Wv = np.asarray(Wv, dtype=np.float32)
    Wp = np.asarray(Wp, dtype=np.float32)
    bq = np.asarray(bq, dtype=np.float32)
    bk = np.asarray(bk, dtype=np.float32)
    bv = np.asarray(bv, dtype=np.float32)

    bf = ml_dtypes.bfloat16
    kk = np.arange(128)[:, None]
    jj = np.arange(128)[None, :]
    tri = (kk <= jj).astype(bf)
    mask2 = np.ascontiguousarray(np.concatenate([tri, tri], axis=1))

    xTs = [np.ascontiguousarray(x[b].T).astype(bf) for b in range(B)]
    gslices = [slice(0, CG), slice(CG, C)]
    in_maps = []
    for core in range(NCORES):
        b, g = core // 2, core % 2
        gs = gslices[g]
        in_maps.append({
            "xT": xTs[b],
            "wqT": np.ascontiguousarray(Wq[gs, :].T).astype(bf),
            "wkT": np.ascontiguousarray(Wk[gs, :].T).astype(bf),
            "wvT": np.ascontiguousarray(Wv[gs, :].T).astype(bf),
            "wpT": np.ascontiguousarray(Wp[:, gs].T).astype(bf),
            "bq2": np.ascontiguousarray(bq[gs].reshape(NHP, 128).T),
            "bk2": np.ascontiguousarray(bk[gs].reshape(NHP, 128).T),
            "bv2": np.ascontiguousarray(bv[gs].reshape(1, CG)).astype(bf),
            "mask": mask2,
        })
    return in_maps


def kernel(x, Wq, bq, Wk, bk, Wv, bv, Wp, bp, **_ignored):
    global last_result
    bp = np.asarray(bp, dtype=np.float32)
    in_maps = prep_in_maps(x, Wq, bq, Wk, bk, Wv, bv, Wp, bp)

    if "nc" not in _compiled:
        _compiled["nc"] = _build()
    nc = _compiled["nc"]

    last_result = bass_utils.run_bass_kernel_spmd(
        nc, in_maps, core_ids=list(range(NCORES)))

    out = np.empty((B, T, C), dtype=np.float32)
    for b in range(B):
        out[b] = last_result.results[2 * b]["out"]
        out[b] += last_result.results[2 * b + 1]["out"]
    out += bp[None, None, :]
    return out
